# revision 56
# baseline (speedup 1.0000x reference)
"""Trainium2 Bass kernel for AttentionPooling (ragged span attention pooling).

Math restructuring (vs the reference's gather-then-project):
  - K/V projections are computed once per unique token (B*T=1024 rows), not per
    gathered span token (B*S*L=131072 rows).
  - The query is a single shared vector, so per-span softmax factorizes:
        attn[s,n,l] = e[start_s+l, n] / Z[s,n],   e[t,n] = exp(q_n . k_{t,n} / 8)
        Z[s,n]      = sum_{t in span_s} e[t,n]
    Hence  ctx[s] = (1/Z[s]) * sum_t W[t,s] * (e[t] (x) V[t])  with the SAME 0/1
    banded window matrix W for all heads -> one dense matmul per core.
  - Everything runs feature-major (feature dim on partitions, spans on the free
    dim) so no on-device transposes are needed anywhere.
  - Attention path in fp32r (full-rate fp32), FFN weights/activations in bf16
    (fp32 PSUM accumulation); the x1 residual into LN2 stays fp32r.

Sharding: flattened (B*S)=4096 spans split over 8 cores (512 each); cores 0-3
serve batch 0, cores 4-7 batch 1. Weights replicated.
"""

import sys
import numpy as np

if "/opt/trn_rl_repo" not in sys.path:
    sys.path.insert(0, "/opt/trn_rl_repo")

B, T, S, H, L, NH = 2, 512, 2048, 768, 32, 12
HD = H // NH            # 64
INTERMED = 4 * H        # 3072
NCORES = 8
SPC = (B * S) // NCORES  # 512 spans per core
HC = H // 128            # 6 feature chunks
IC = INTERMED // 128     # 24 intermediate chunks
TC = T // 128            # 4 token chunks
SC = SPC // 128          # 4 span chunks
VW = H + NH              # 780: [e-scaled V | e]
WG = 4                   # i-chunks per streamed weight group
NG = IC // WG            # 6 weight groups
EPS = 1e-5

_COMPILED = None


def _build(gb_identity=False):
    import concourse.bacc as bacc
    import concourse.tile as tile
    from concourse import mybir
    from concourse.alu_op_type import AluOpType as Op

    f32 = mybir.dt.float32
    f32r = mybir.dt.float32r
    bf16 = mybir.dt.bfloat16
    Act = mybir.ActivationFunctionType

    nc = bacc.Bacc("TRN2", target_bir_lowering=False, debug=False, num_devices=NCORES)

    def din(name, shape, dt=f32):
        return nc.dram_tensor(name, list(shape), dt, kind="ExternalInput").ap()

    xT = din("xT", [H, T], bf16)      # (x = token_reps + pe), transposed
    wvl = din("wvl", [H, VW], bf16)   # [Wv.T | wq2.T]
    bvl = din("bvl", [1, VW], bf16)   # [bv | q.bk per head]
    starts = din("starts", [1, SPC])  # span starts (f32)
    ends = din("ends", [1, SPC])      # span start + len*mask (f32)
    iot = din("iota", [128, TC])      # t_global per (partition, t-chunk)
    ssel = din("ssel", [NH, H])       # head selector: ssel[n,h'] = (h'//64 == n)
    wout = din("wout", [H, H], bf16)  # (Wout - colmean(Wout)).T  [h', h]
    b1c = din("b1c", [H])             # centered (out_b + query) bias for LN1
    gco = din("gco", [H])             # norm gamma
    bco = din("bco", [H])             # norm beta
    w1t = din("w1t", [NG, 128, HC, WG * 128], bf16)  # packed W1.T groups
    b1r = din("b1r", [INTERMED])      # ffn_b1
    w2t = din("w2t", [NG, 128, WG, H], bf16)         # packed W2.T groups
    b2r = din("b2r", [H])             # ffn_b2
    maskc = din("maskc", [SPC])       # span mask (f32)
    i128 = din("i128", [128, 128])    # identity
    onesv = din("onesv", [128])       # ones (f32r matmul operand source)
    onesb = din("onesb", [128], bf16)  # ones (bf16)

    out = nc.dram_tensor("out", [SPC, H], f32, kind="ExternalOutput").ap()

    def r(ap):
        return ap.bitcast(f32r)

    with tile.TileContext(nc) as tc:
        with (
            tc.tile_pool(name="consts", bufs=1) as cp,
            tc.tile_pool(name="x1keep", bufs=1) as x1p,
            tc.tile_pool(name="w1s", bufs=3) as w1p,
            tc.tile_pool(name="w2s", bufs=6) as w2p,
        ):
            ones1 = cp.tile([1, 128], f32r)      # K=1 matmul lhsT
            nc.gpsimd.dma_start(ones1[:], onesv.unsqueeze(0).bitcast(f32r))
            ones1b = cp.tile([1, 128], bf16)
            nc.gpsimd.dma_start(ones1b[:], onesb.unsqueeze(0))
            bvl_sb = cp.tile([1, VW], bf16)
            nc.gpsimd.dma_start(bvl_sb[:], bvl)
            starts_r = cp.tile([1, SPC], f32)
            nc.gpsimd.dma_start(starts_r[:], starts)
            ends_r = cp.tile([1, SPC], f32)
            nc.gpsimd.dma_start(ends_r[:], ends)
            iota_sb = cp.tile([128, TC], f32)
            nc.gpsimd.dma_start(iota_sb[:], iot)
            eps1 = cp.tile([1, 1], f32)
            nc.vector.memset(eps1, EPS)
            eps128 = cp.tile([128, 1], f32)
            nc.vector.memset(eps128, EPS)
            zero128 = cp.tile([128, 1], f32)
            nc.vector.memset(zero128, 0.0)
            onescol = cp.tile([128, 1], f32r)    # partition-colsum lhsT
            nc.gpsimd.dma_start(onescol[:], onesv.unsqueeze(1).bitcast(f32r))
            i128_sb = cp.tile([128, 128], f32r)
            nc.gpsimd.dma_start(i128_sb[:], i128.bitcast(f32r))
            gcol = cp.tile([128, HC], f32)      # gamma as per-partition cols
            nc.gpsimd.dma_start(gcol[:], gco.rearrange("(c p) -> p c", p=128))
            bcol = cp.tile([128, HC], f32)
            nc.gpsimd.dma_start(bcol[:], bco.rearrange("(c p) -> p c", p=128))
            b1ccol = cp.tile([128, HC], f32)
            nc.gpsimd.dma_start(b1ccol[:], b1c.rearrange("(c p) -> p c", p=128))
            b1col = cp.tile([128, IC], f32)
            nc.gpsimd.dma_start(b1col[:], b1r.rearrange("(c p) -> p c", p=128))
            maskcol = cp.tile([128, SC], f32)
            nc.gpsimd.dma_start(maskcol[:], maskc.rearrange("(c p) -> p c", p=128))
            ssel_sb = cp.tile([NH, H], f32r)
            nc.gpsimd.dma_start(ssel_sb[:], ssel.bitcast(f32r))

            x1T = x1p.tile([128, HC, SPC], f32r)   # LN1 output, feature-major
            x1Tb = x1p.tile([128, HC, SPC], bf16)  # bf16 copy for FFN1 rhs

            # FFN weight tiles allocated early (stable addresses); DMAs are
            # issued later so attention-critical loads win the queue order.
            w1g = [w1p.tile([128, HC, WG * 128], bf16, tag="w1", name=f"w1g{g}")
                   for g in range(3)]
            w2g = [w2p.tile([128, WG, H], bf16, tag="w2", name=f"w2g{g}")
                   for g in range(NG)]

            # ---------------- attention (feature-major) ----------------
            with (
                tc.tile_pool(name="attn", bufs=1) as ap_,
                tc.tile_pool(name="attn_s", bufs=2) as asml,
                tc.tile_pool(name="psA", bufs=2, space="PSUM") as psA,
                tc.tile_pool(name="psB", bufs=2, space="PSUM") as psB,
                tc.tile_pool(name="psU", bufs=1, space="PSUM") as psU,
                tc.tile_pool(name="psC", bufs=1, space="PSUM") as psC,
            ):
                xTc = [ap_.tile([128, T], bf16, name=f"xTc{c}")
                       for c in range(HC)]
                wvlc = [ap_.tile([128, VW], bf16, name=f"wvlc{c}")
                        for c in range(HC)]
                xTr = xT.rearrange("(c p) t -> c p t", p=128)
                wvlr = wvl.rearrange("(c p) n -> c p n", p=128)
                # warm up the PE (HAM clock gate) while the first loads
                # land; memset-sourced fp32 operands need no DMA
                wf = asml.tile([1, 128], f32, tag="wf")
                nc.vector.memset(wf, 1.0)
                dum = psC.tile([128, 128], f32, tag="small")
                for k in range(14):
                    nc.tensor.matmul(dum[:], wf[:], wf[:],
                                     start=(k == 0), stop=(k == 13))
                # broadcast span starts/ends across partitions on the PE
                # (cheaper than a 128x-replicating DMA)
                startsB = psU.tile([128, SPC], f32, tag="up", name="startsB")
                nc.tensor.matmul(startsB[:], wf[:], starts_r[:],
                                 start=True, stop=True)
                endsB = psC.tile([128, SPC], f32, tag="small", name="endsB")
                nc.tensor.matmul(endsB[:], wf[:], ends_r[:],
                                 start=True, stop=True)
                # per-chunk loads round-robin over three queues so the first
                # V' matmul starts ASAP (DMA triggers serialize per queue)
                qs = [nc.sync, nc.scalar]
                for c in range(HC):
                    qs[0].dma_start(wvlc[c][:], wvlr[c])
                    qs[1].dma_start(xTc[c][:], xTr[c])
                wout_sb = ap_.tile([128, HC, H], bf16)
                nc.sync.dma_start(wout_sb[:], wout.rearrange("(c p) n -> p c n", p=128))
                # now the FFN weight prefetch (fills DMA idle during attention)
                for g in range(3):
                    nc.sync.dma_start(w1g[g][:], w1t[g])
                    nc.scalar.dma_start(w2g[g][:], w2t[g])

                ve = ap_.tile([128, TC, VW], bf16)    # [e*V | e], token-major
                wt = ap_.tile([128, TC, SPC], bf16)   # W[t, s] 0/1 window matrix
                ctxN = ap_.tile([128, HC, SPC], bf16) # normalized ctx

                # V' = x @ [Wv.T | wq2.T] + bias  (out: token-major, 780 wide)
                for t in range(TC):
                    vp = psA.tile([128, VW], f32, tag="vp")
                    for lo, hi in ((0, 512), (512, VW)):
                        for c in range(HC):
                            nc.tensor.matmul(
                                vp[:, lo:hi],
                                xTc[c][:, t * 128:(t + 1) * 128],
                                wvlc[c][:, lo:hi],
                                start=(c == 0), stop=False,
                            )
                        nc.tensor.matmul(
                            vp[:, lo:hi], ones1b[:], bvl_sb[:, lo:hi],
                            start=False, stop=True,
                        )
                    # e = exp(logits) into ve[:, t, 768:780]
                    nc.scalar.activation(ve[:, t, H:VW], vp[:, H:VW], Act.Exp)
                    # ve[:, t, :768] = V * e (per-head broadcast of e over 64 cols)
                    e_b = ve[:, t, H:VW].unsqueeze(2).broadcast_to([128, NH, HD])
                    nc.vector.tensor_tensor(
                        ve[:, t, 0:H].rearrange("p (n d) -> p n d", d=HD),
                        vp[:, 0:H].rearrange("p (n d) -> p n d", d=HD),
                        e_b, Op.mult,
                    )
                    # W[t, s] = (start_s <= t) & (t < end_s)
                    lt = asml.tile([128, SPC], f32, tag="uexp", name="lt")
                    nc.vector.tensor_scalar(
                        lt[:], endsB[:], iota_sb[:, t:t + 1], None, Op.is_gt)
                    nc.vector.scalar_tensor_tensor(
                        wt[:, t, :], startsB[:], iota_sb[:, t:t + 1], lt[:],
                        Op.is_le, Op.mult)

                # Z[n, s] then u = 1/max(Z, tiny)
                zp = psC.tile([NH, SPC], f32, tag="small")
                for t in range(TC):
                    nc.tensor.matmul(
                        zp[:], ve[:, t, H:VW], wt[:, t, :],
                        start=(t == 0), stop=(t == TC - 1))
                zc = asml.tile([NH, SPC], f32, tag="zc")
                nc.vector.tensor_scalar(zc[:], zp[:], 1e-6, None, Op.max)
                uf = asml.tile([NH, SPC], f32, tag="uexp", name="uf")
                us = asml.tile([NH, SPC], f32, tag="sq", name="us")
                nc.vector.reciprocal_approx_accurate(uf[:], zc[:], us[:])
                u_sb = asml.tile([NH, SPC], f32r, tag="u")
                with nc.allow_low_precision(reason="f32r is fp32-width"):
                    nc.vector.tensor_copy(u_sb[:], uf[:])

                # ctxN[h', s] = (sum_t ve[t, h'] * W[t, s]) * u[head(h'), s]
                for c in range(HC):
                    up = psU.tile([128, SPC], f32, tag="up")
                    nc.tensor.matmul(
                        up[:], ssel_sb[:, c * 128:(c + 1) * 128], u_sb[:],
                        start=True, stop=True)
                    uexp = asml.tile([128, SPC], f32, tag="uexp")
                    nc.scalar.activation(uexp[:], up[:], Act.Identity)
                    cp_ = psB.tile([128, SPC], f32, tag="mm512")
                    for t in range(TC):
                        nc.tensor.matmul(
                            cp_[:], ve[:, t, c * 128:(c + 1) * 128],
                            wt[:, t, :],
                            start=(t == 0), stop=(t == TC - 1))
                    nc.vector.tensor_tensor(ctxN[:, c, :], cp_[:], uexp[:], Op.mult)

                # out_proj (centered weights) + LN1, feature-major
                ycs = ap_.tile([128, HC, SPC], f32)
                varp = psC.tile([1, SPC], f32, tag="small")
                for m in range(HC):
                    aop = psB.tile([128, SPC], f32, tag="mm512")
                    for c in range(HC):
                        nc.tensor.matmul(
                            aop[:], wout_sb[:, c, m * 128:(m + 1) * 128],
                            ctxN[:, c, :],
                            start=(c == 0), stop=(c == HC - 1))
                    # yc = aop + centered bias (per-partition) on ACT
                    nc.scalar.activation(ycs[:, m, :], aop[:], Act.Identity,
                                         bias=b1ccol[:, m:m + 1])
                for m in range(HC):
                    sq = asml.tile([128, SPC], f32r, tag="sq")
                    with nc.allow_low_precision(reason="f32r is fp32-width"):
                        nc.vector.tensor_tensor(sq[:], ycs[:, m, :],
                                                ycs[:, m, :], Op.mult)
                    nc.tensor.matmul(
                        varp[:], onescol[:], sq[:],
                        start=(m == 0), stop=(m == HC - 1))
                sd = asml.tile([1, SPC], f32, tag="sd")
                nc.scalar.activation(sd[:], varp[:], Act.Sqrt,
                                     bias=eps1[:], scale=1.0 / H)
                rf = asml.tile([1, SPC], f32, tag="uexp", name="rf")
                rs_ = asml.tile([1, SPC], f32, tag="sq", name="rs_")
                nc.vector.reciprocal_approx_accurate(rf[:], sd[:], rs_[:])
                rstd = asml.tile([1, SPC], f32r, tag="rstd")
                with nc.allow_low_precision(reason="f32r is fp32-width"):
                    nc.vector.tensor_copy(rstd[:], rf[:])
                rp = psC.tile([128, SPC], f32, tag="small")
                nc.tensor.matmul(rp[:], ones1[:], rstd[:],
                                 start=True, stop=True)
                for m in range(HC):
                    if gb_identity:
                        with nc.allow_low_precision(reason="fp32-width"):
                            nc.vector.tensor_tensor(x1T[:, m, :], ycs[:, m, :],
                                                    rp[:], Op.mult)
                    else:
                        tmp = asml.tile([128, SPC], f32, tag="uexp", name="tmp")
                        nc.vector.tensor_tensor(tmp[:], ycs[:, m, :], rp[:],
                                                Op.mult)
                        # x1 = tmp * g + b (per-partition scalars) on ACT
                        nc.scalar.activation(x1T[:, m, :], tmp[:], Act.Identity,
                                             scale=gcol[:, m:m + 1],
                                             bias=bcol[:, m:m + 1])
                    nc.vector.tensor_copy(x1Tb[:, m, :], x1T[:, m, :].bitcast(f32))

            # ---------------- FFN + LN2 (h2 span-major) ----------------
            with (
                tc.tile_pool(name="ffn", bufs=1) as fp_,
                tc.tile_pool(name="outp", bufs=4) as op_,
            ):
                gB = fp_.tile([128, H], f32)    # gamma broadcast along partitions
                nc.gpsimd.dma_start(gB[:], gco.unsqueeze(0).broadcast_to([128, H]))
                bB = fp_.tile([128, H], f32)
                nc.gpsimd.dma_start(bB[:], bco.unsqueeze(0).broadcast_to([128, H]))
                b2B = fp_.tile([128, H], f32)
                nc.gpsimd.dma_start(b2B[:], b2r.unsqueeze(0).broadcast_to([128, H]))

                h1s = fp_.tile([128, IC, SPC], bf16)
                for g in range(3, NG):  # rest of W2 lands during FFN1
                    nc.scalar.dma_start(w2g[g][:], w2t[g])
                psZcm = tc.tile_pool(name="psZA", bufs=1, space="PSUM")
                psZ = psZcm.__enter__()  # wave-A banks held alongside psH
                with tc.tile_pool(name="psH", bufs=2, space="PSUM") as psH:
                    for i in range(IC):
                        g, l = divmod(i, WG)
                        if l == 0 and g >= 3:  # stream remaining weight groups
                            w1g[g % 3] = w1p.tile([128, HC, WG * 128], bf16,
                                                  tag="w1", name=f"w1g{g}")
                            nc.sync.dma_start(w1g[g % 3][:], w1t[g])
                        w1 = w1g[g % 3]
                        h1p = psH.tile([128, SPC], f32, tag="h1p")
                        for c in range(HC):
                            nc.tensor.matmul(
                                h1p[:], w1[:, c, l * 128:(l + 1) * 128],
                                x1Tb[:, c, :],
                                start=(c == 0), stop=(c == HC - 1))
                        nc.scalar.activation(h1s[:, i, :], h1p[:], Act.Relu,
                                             bias=b1col[:, i:i + 1])

                def residual_first(sc, h2p):
                    # residual: += x1 (identity matmuls transpose x1T blocks).
                    # Runs BEFORE the W2 accumulation (start=True) so no
                    # PE work remains between the last W2 matmul and LN2.
                    for hc in range(HC):
                        # start=True once per PSUM bank (clears has_written)
                        nc.tensor.matmul(
                            h2p[:, hc * 128:(hc + 1) * 128],
                            x1T[:, hc, sc * 128:(sc + 1) * 128],
                            i128_sb[:],
                            start=(hc == 0 or hc == 4), stop=False)

                def ln2_stats(sc, h2p):
                    z = op_.tile([128, H], f32, tag="z", name=f"z{sc}")
                    nc.vector.tensor_tensor(z[:], h2p[:], b2B[:], Op.add)
                    stats = op_.tile([128, 3, 6], f32, tag="stats",
                                     name=f"st{sc}")
                    zg = z[:].rearrange("p (g d) -> p g d", g=3)
                    for g in range(3):
                        nc.vector.bn_stats(stats[:, g, :], zg[:, g, :])
                    mv = op_.tile([128, 2], f32, tag="mv", name=f"mv{sc}")
                    nc.vector.bn_aggr(mv[:], stats[:])
                    return z, mv

                def ln2_sqrt(sc, mv):
                    sd2 = op_.tile([128, 1], f32, tag="sd2", name=f"sd2{sc}")
                    nc.scalar.activation(sd2[:], mv[:, 1:2], Act.Sqrt,
                                         bias=eps128[:])
                    rstd2 = op_.tile([128, 1], f32, tag="rstd2",
                                     name=f"rs2{sc}")
                    nc.vector.reciprocal(rstd2[:], sd2[:])
                    negmr = op_.tile([128, 1], f32, tag="negmr",
                                     name=f"nm{sc}")
                    nc.vector.scalar_tensor_tensor(
                        negmr[:], mv[:, 0:1], -1.0, rstd2[:], Op.mult, Op.mult)
                    return rstd2, negmr

                def ln2_norm(sc, z, rstd2, negmr):
                    if gb_identity:
                        # fold the mask into the rstd/bias scalars: one ACT op
                        rsm = op_.tile([128, 1], f32, tag="rsm", name=f"rsm{sc}")
                        nc.vector.tensor_scalar(
                            rsm[:], rstd2[:], maskcol[:, sc:sc + 1], None,
                            Op.mult)
                        nmm = op_.tile([128, 1], f32, tag="nmm", name=f"nmm{sc}")
                        nc.vector.tensor_scalar(
                            nmm[:], negmr[:], maskcol[:, sc:sc + 1], None,
                            Op.mult)
                        o = op_.tile([128, H], f32, tag="o", name=f"o{sc}")
                        nc.scalar.activation(o[:], z[:], Act.Identity,
                                             scale=rsm[:], bias=nmm[:])
                    else:
                        zn = op_.tile([128, H], f32, tag="zn", name=f"zn{sc}")
                        nc.scalar.activation(zn[:], z[:], Act.Identity,
                                             scale=rstd2[:], bias=negmr[:])
                        zn2 = op_.tile([128, H], f32, tag="zn2", name=f"zn2{sc}")
                        nc.vector.tensor_tensor(zn2[:], zn[:], gB[:], Op.mult)
                        zn3 = op_.tile([128, H], f32, tag="zn3", name=f"zn3{sc}")
                        nc.vector.tensor_tensor(zn3[:], zn2[:], bB[:], Op.add)
                        o = op_.tile([128, H], f32, tag="o", name=f"o{sc}")
                        nc.scalar.activation(o[:], zn3[:], Act.Identity,
                                             scale=maskcol[:, sc:sc + 1],
                                             bias=zero128[:])
                    nc.sync.dma_start(out[sc * 128:(sc + 1) * 128, :], o[:])

                # two waves of 2 span-chunks: wave A's matmuls interleave with
                # the FFN1 tail (4 PSUM banks fit alongside psH), and wave A's
                # LN2 overlaps wave B's matmuls
                psZBcm = tc.tile_pool(name="psZB", bufs=1, space="PSUM")
                psZB = psZBcm.__enter__()
                for wave in range(2):
                    scs = (wave * 2, wave * 2 + 1)
                    pool = psZ if wave == 0 else psZB
                    h2ps = {sc: pool.tile([128, H], f32, tag=f"h2_{sc}",
                                          name=f"h2p{sc}") for sc in scs}
                    for sc in scs:
                        residual_first(sc, h2ps[sc])
                    for i in range(IC):
                        g, l = divmod(i, WG)
                        w2 = w2g[g]
                        for sc in scs:
                            for lo, hi in ((0, 512), (512, H)):
                                nc.tensor.matmul(
                                    h2ps[sc][:, lo:hi],
                                    h1s[:, i, sc * 128:(sc + 1) * 128],
                                    w2[:, l, lo:hi],
                                    start=False, stop=(i == IC - 1))
                    zmv = {sc: ln2_stats(sc, h2ps[sc]) for sc in scs}
                    rn = {sc: ln2_sqrt(sc, zmv[sc][1]) for sc in scs}
                    for sc in scs:
                        ln2_norm(sc, zmv[sc][0], rn[sc][0], rn[sc][1])
                psZBcm.__exit__(None, None, None)
                psZcm.__exit__(None, None, None)
    nc.compile()
    return nc


def _host_prepare(inputs):
    """Host-side packing: tiny index/weight reshapes, no heavy math."""
    import ml_dtypes
    bf = ml_dtypes.bfloat16

    tr = np.asarray(inputs["token_reps"], dtype=np.float32)
    span_ids = np.asarray(inputs["span_ids"]).astype(np.int64)
    masks = np.asarray(inputs["span_masks"]).astype(np.float32)
    pe = np.asarray(inputs["pe"], dtype=np.float32)
    q0 = np.asarray(inputs["dummy_query"], dtype=np.float32)
    in_w = np.asarray(inputs["in_proj_w"], dtype=np.float32)
    in_b = np.asarray(inputs["in_proj_b"], dtype=np.float32)
    wo = np.asarray(inputs["out_proj_w"], dtype=np.float32)
    bo = np.asarray(inputs["out_proj_b"], dtype=np.float32)
    g = np.asarray(inputs["norm_g"], dtype=np.float32)
    bb = np.asarray(inputs["norm_b"], dtype=np.float32)
    w1 = np.asarray(inputs["ffn_w1"], dtype=np.float32)
    b1 = np.asarray(inputs["ffn_b1"], dtype=np.float32)
    w2 = np.asarray(inputs["ffn_w2"], dtype=np.float32)
    b2 = np.asarray(inputs["ffn_b2"], dtype=np.float32)

    Wq, Wk, Wv = in_w[0:H], in_w[H:2 * H], in_w[2 * H:3 * H]
    bq, bk, bv = in_b[0:H], in_b[H:2 * H], in_b[2 * H:3 * H]

    q = q0 @ Wq.T + bq
    qs = (q / np.sqrt(HD)).astype(np.float32)
    wq2 = np.stack([qs[n * HD:(n + 1) * HD] @ Wk[n * HD:(n + 1) * HD]
                    for n in range(NH)])                      # (12, 768)
    constv = np.array([qs[n * HD:(n + 1) * HD] @ bk[n * HD:(n + 1) * HD]
                       for n in range(NH)], dtype=np.float32)

    wvl = np.concatenate([Wv.T, wq2.T], axis=1).astype(np.float32)   # (768, 780)
    bvl = np.concatenate([bv, constv])[None, :].astype(np.float32)   # (1, 780)

    wout_c = wo - wo.mean(axis=0, keepdims=True)
    wout_ct = np.ascontiguousarray(wout_c.T).astype(np.float32)      # (768, 768)
    b1c_full = bo + q0
    b1c = (b1c_full - b1c_full.mean()).astype(np.float32)

    ssel = np.zeros((NH, H), dtype=np.float32)
    for n in range(NH):
        ssel[n, n * HD:(n + 1) * HD] = 1.0
    iota = (np.arange(128, dtype=np.float32)[:, None]
            + 128.0 * np.arange(TC, dtype=np.float32)[None, :])
    iota = np.ascontiguousarray(iota)
    i128 = np.eye(128, dtype=np.float32)

    # packed bf16 FFN weights:
    # w1t[g, p, c, l*128+n] = W1.T[c*128+p, (g*WG+l)*128+n]
    w1T = w1.T.reshape(HC, 128, NG, WG * 128)          # (c, p, g, l*128+n)
    w1tp = np.ascontiguousarray(
        w1T.transpose(2, 1, 0, 3)).astype(bf)          # (g, p, c, 512)
    # w2t[g, p, l, h] = W2.T[(g*WG+l)*128+p, h]
    w2T = w2.T.reshape(NG, WG, 128, H)                 # (g, l, p, h)
    w2tp = np.ascontiguousarray(
        w2T.transpose(0, 2, 1, 3)).astype(bf)          # (g, p, l, h)

    x = tr + pe[None, :T]                              # (B, T, H)
    xTs = [np.ascontiguousarray(x[b].T).astype(np.float32) for b in range(B)]

    starts_all = span_ids[..., 0].astype(np.float32)                      # (B, S)
    lens_all = (span_ids[..., 1] - span_ids[..., 0]).astype(np.float32) * masks
    ends_all = starts_all + lens_all

    shared = dict(wvl=wvl.astype(bf), bvl=bvl.astype(bf), iota=iota,
                  ssel=ssel, wout=wout_ct.astype(bf),
                  b1c=b1c, gco=g, bco=bb, w1t=w1tp, b1r=b1, w2t=w2tp, b2r=b2,
                  i128=i128, onesv=np.ones(128, dtype=np.float32),
                  onesb=np.ones(128, dtype=bf))
    in_maps = []
    for core in range(NCORES):
        b = core // (NCORES // B)
        s0 = (core % (NCORES // B)) * SPC
        m = dict(shared)
        m["xT"] = xTs[b].astype(bf)
        m["starts"] = np.ascontiguousarray(starts_all[b, s0:s0 + SPC][None, :])
        m["ends"] = np.ascontiguousarray(ends_all[b, s0:s0 + SPC][None, :])
        m["maskc"] = np.ascontiguousarray(masks[b, s0:s0 + SPC])
        in_maps.append(m)
    return in_maps


def kernel(**inputs) -> np.ndarray:
    global _COMPILED
    from concourse.bass_utils import run_bass_kernel_spmd

    if _COMPILED is None:
        gbi = (np.allclose(np.asarray(inputs["norm_g"], dtype=np.float32), 1.0)
               and np.allclose(np.asarray(inputs["norm_b"], dtype=np.float32),
                               0.0))
        _COMPILED = _build(gb_identity=gbi)
    nc = _COMPILED
    in_maps = _host_prepare(inputs)
    res = run_bass_kernel_spmd(nc, in_maps, core_ids=list(range(NCORES)))
    outs = [res.results[i]["out"] for i in range(NCORES)]
    full = np.concatenate(outs, axis=0).reshape(B, S, H)
    return full.astype(np.float32)


# revision 57
# speedup vs baseline: 1.0196x; 1.0196x over previous
"""Trainium2 Bass kernel for AttentionPooling (ragged span attention pooling).

Math restructuring (vs the reference's gather-then-project):
  - K/V projections are computed once per unique token (B*T=1024 rows), not per
    gathered span token (B*S*L=131072 rows).
  - The query is a single shared vector, so per-span softmax factorizes:
        attn[s,n,l] = e[start_s+l, n] / Z[s,n],   e[t,n] = exp(q_n . k_{t,n} / 8)
        Z[s,n]      = sum_{t in span_s} e[t,n]
    Hence  ctx[s] = (1/Z[s]) * sum_t W[t,s] * (e[t] (x) V[t])  with the SAME 0/1
    banded window matrix W for all heads -> one dense matmul per core.
  - Everything runs feature-major (feature dim on partitions, spans on the free
    dim) so no on-device transposes are needed anywhere.
  - Attention path in fp32r (full-rate fp32), FFN weights/activations in bf16
    (fp32 PSUM accumulation); the x1 residual into LN2 stays fp32r.

Sharding: flattened (B*S)=4096 spans split over 8 cores (512 each); cores 0-3
serve batch 0, cores 4-7 batch 1. Weights replicated.
"""

import sys
import numpy as np

if "/opt/trn_rl_repo" not in sys.path:
    sys.path.insert(0, "/opt/trn_rl_repo")

B, T, S, H, L, NH = 2, 512, 2048, 768, 32, 12
HD = H // NH            # 64
INTERMED = 4 * H        # 3072
NCORES = 8
SPC = (B * S) // NCORES  # 512 spans per core
HC = H // 128            # 6 feature chunks
IC = INTERMED // 128     # 24 intermediate chunks
TC = T // 128            # 4 token chunks
SC = SPC // 128          # 4 span chunks
VW = H + NH              # 780: [e-scaled V | e]
WG = 4                   # i-chunks per streamed weight group
NG = IC // WG            # 6 weight groups
EPS = 1e-5

_COMPILED = None


def _build(gb_identity=False):
    import concourse.bacc as bacc
    import concourse.tile as tile
    from concourse import mybir
    from concourse.alu_op_type import AluOpType as Op

    f32 = mybir.dt.float32
    f32r = mybir.dt.float32r
    bf16 = mybir.dt.bfloat16
    Act = mybir.ActivationFunctionType

    nc = bacc.Bacc("TRN2", target_bir_lowering=False, debug=False, num_devices=NCORES)

    def din(name, shape, dt=f32):
        return nc.dram_tensor(name, list(shape), dt, kind="ExternalInput").ap()

    xT = din("xT", [H, T], bf16)      # (x = token_reps + pe), transposed
    wvl = din("wvl", [H, VW], bf16)   # [Wv.T | wq2.T]
    bvl = din("bvl", [1, VW], bf16)   # [bv | q.bk per head]
    starts = din("starts", [1, SPC])  # span starts (f32)
    ends = din("ends", [1, SPC])      # span start + len*mask (f32)
    iot = din("iota", [128, TC])      # t_global per (partition, t-chunk)
    ssel = din("ssel", [NH, H])       # head selector: ssel[n,h'] = (h'//64 == n)
    wout = din("wout", [H, H], bf16)  # (Wout - colmean(Wout)).T  [h', h]
    b1c = din("b1c", [H])             # centered (out_b + query) bias for LN1
    gco = din("gco", [H])             # norm gamma
    bco = din("bco", [H])             # norm beta
    w1t = din("w1t", [NG, 128, HC, WG * 128], bf16)  # packed W1.T groups
    b1r = din("b1r", [INTERMED])      # ffn_b1
    w2t = din("w2t", [NG, 128, WG, H], bf16)         # packed W2.T groups
    b2r = din("b2r", [H])             # ffn_b2
    maskc = din("maskc", [SPC])       # span mask (f32)
    i128 = din("i128", [128, 128])    # identity
    onesv = din("onesv", [128])       # ones (f32r matmul operand source)
    onesb = din("onesb", [128], bf16)  # ones (bf16)

    out = nc.dram_tensor("out", [SPC, H], f32, kind="ExternalOutput").ap()

    def r(ap):
        return ap.bitcast(f32r)

    with tile.TileContext(nc) as tc:
        with (
            tc.tile_pool(name="consts", bufs=1) as cp,
            tc.tile_pool(name="x1keep", bufs=1) as x1p,
            tc.tile_pool(name="w1s", bufs=3) as w1p,
            tc.tile_pool(name="w2s", bufs=6) as w2p,
        ):
            ones1 = cp.tile([1, 128], f32r)      # K=1 matmul lhsT
            nc.gpsimd.dma_start(ones1[:], onesv.unsqueeze(0).bitcast(f32r))
            ones1b = cp.tile([1, 128], bf16)
            nc.gpsimd.dma_start(ones1b[:], onesb.unsqueeze(0))
            bvl_sb = cp.tile([1, VW], bf16)
            nc.gpsimd.dma_start(bvl_sb[:], bvl)
            starts_r = cp.tile([1, SPC], f32)
            nc.gpsimd.dma_start(starts_r[:], starts)
            ends_r = cp.tile([1, SPC], f32)
            nc.gpsimd.dma_start(ends_r[:], ends)
            iota_sb = cp.tile([128, TC], f32)
            nc.gpsimd.dma_start(iota_sb[:], iot)
            eps1 = cp.tile([1, 1], f32)
            nc.vector.memset(eps1, EPS)
            eps128 = cp.tile([128, 1], f32)
            nc.vector.memset(eps128, EPS)
            zero128 = cp.tile([128, 1], f32)
            nc.vector.memset(zero128, 0.0)
            onescol = cp.tile([128, 1], f32r)    # partition-colsum lhsT
            nc.gpsimd.dma_start(onescol[:], onesv.unsqueeze(1).bitcast(f32r))
            i128_sb = cp.tile([128, 128], f32r)
            nc.gpsimd.dma_start(i128_sb[:], i128.bitcast(f32r))
            gcol = cp.tile([128, HC], f32)      # gamma as per-partition cols
            nc.gpsimd.dma_start(gcol[:], gco.rearrange("(c p) -> p c", p=128))
            bcol = cp.tile([128, HC], f32)
            nc.gpsimd.dma_start(bcol[:], bco.rearrange("(c p) -> p c", p=128))
            b1ccol = cp.tile([128, HC], f32)
            nc.gpsimd.dma_start(b1ccol[:], b1c.rearrange("(c p) -> p c", p=128))
            b1col = cp.tile([128, IC], f32)
            nc.gpsimd.dma_start(b1col[:], b1r.rearrange("(c p) -> p c", p=128))
            maskcol = cp.tile([128, SC], f32)
            nc.gpsimd.dma_start(maskcol[:], maskc.rearrange("(c p) -> p c", p=128))
            ssel_sb = cp.tile([NH, H], f32r)
            nc.gpsimd.dma_start(ssel_sb[:], ssel.bitcast(f32r))

            x1T = x1p.tile([128, HC, SPC], f32r)   # LN1 output, feature-major
            x1Tb = x1p.tile([128, HC, SPC], bf16)  # bf16 copy for FFN1 rhs

            # FFN weight tiles allocated early (stable addresses); DMAs are
            # issued later so attention-critical loads win the queue order.
            w1g = [w1p.tile([128, HC, WG * 128], bf16, tag="w1", name=f"w1g{g}")
                   for g in range(3)]
            w2g = [w2p.tile([128, WG, H], bf16, tag="w2", name=f"w2g{g}")
                   for g in range(NG)]

            # ---------------- attention (feature-major) ----------------
            with (
                tc.tile_pool(name="attn", bufs=1) as ap_,
                tc.tile_pool(name="attn_s", bufs=2) as asml,
                tc.tile_pool(name="psA", bufs=2, space="PSUM") as psA,
                tc.tile_pool(name="psB", bufs=2, space="PSUM") as psB,
                tc.tile_pool(name="psU", bufs=1, space="PSUM") as psU,
                tc.tile_pool(name="psC", bufs=1, space="PSUM") as psC,
            ):
                xTc = [ap_.tile([128, T], bf16, name=f"xTc{c}")
                       for c in range(HC)]
                wvlc = [ap_.tile([128, VW], bf16, name=f"wvlc{c}")
                        for c in range(HC)]
                xTr = xT.rearrange("(c p) t -> c p t", p=128)
                wvlr = wvl.rearrange("(c p) n -> c p n", p=128)
                # warm up the PE (HAM clock gate) while the first loads
                # land; memset-sourced fp32 operands need no DMA
                wf = asml.tile([1, 128], f32, tag="wf")
                nc.vector.memset(wf, 1.0)
                dum = psC.tile([128, 128], f32, tag="small")
                for k in range(14):
                    nc.tensor.matmul(dum[:], wf[:], wf[:],
                                     start=(k == 0), stop=(k == 13))
                # broadcast span starts/ends across partitions on the PE
                # (cheaper than a 128x-replicating DMA)
                startsB = psU.tile([128, SPC], f32, tag="up", name="startsB")
                nc.tensor.matmul(startsB[:], wf[:], starts_r[:],
                                 start=True, stop=True)
                endsB = psC.tile([128, SPC], f32, tag="small", name="endsB")
                nc.tensor.matmul(endsB[:], wf[:], ends_r[:],
                                 start=True, stop=True)
                # per-chunk loads round-robin over three queues so the first
                # V' matmul starts ASAP (DMA triggers serialize per queue)
                qs = [nc.sync, nc.scalar]
                for c in range(HC):
                    qs[0].dma_start(wvlc[c][:], wvlr[c])
                    qs[1].dma_start(xTc[c][:], xTr[c])
                wout_sb = ap_.tile([128, HC, H], bf16)
                nc.sync.dma_start(wout_sb[:], wout.rearrange("(c p) n -> p c n", p=128))
                # now the FFN weight prefetch (fills DMA idle during attention)
                for g in range(3):
                    nc.sync.dma_start(w1g[g][:], w1t[g])
                    nc.scalar.dma_start(w2g[g][:], w2t[g])

                ve = ap_.tile([128, TC, VW], bf16)    # [e*V | e], token-major
                wt = ap_.tile([128, TC, SPC], bf16)   # W[t, s] 0/1 window matrix
                ctxN = ap_.tile([128, HC, SPC], bf16) # normalized ctx

                # V' = x @ [Wv.T | wq2.T] + bias  (out: token-major, 780 wide)
                for t in range(TC):
                    vp = psA.tile([128, VW], f32, tag="vp")
                    for lo, hi in ((0, 512), (512, VW)):
                        for c in range(HC):
                            nc.tensor.matmul(
                                vp[:, lo:hi],
                                xTc[c][:, t * 128:(t + 1) * 128],
                                wvlc[c][:, lo:hi],
                                start=(c == 0), stop=False,
                            )
                        nc.tensor.matmul(
                            vp[:, lo:hi], ones1b[:], bvl_sb[:, lo:hi],
                            start=False, stop=True,
                        )
                    # e = exp(logits) into ve[:, t, 768:780]
                    nc.scalar.activation(ve[:, t, H:VW], vp[:, H:VW], Act.Exp)
                    # ve[:, t, :768] = V * e (per-head broadcast of e over 64 cols)
                    e_b = ve[:, t, H:VW].unsqueeze(2).broadcast_to([128, NH, HD])
                    nc.vector.tensor_tensor(
                        ve[:, t, 0:H].rearrange("p (n d) -> p n d", d=HD),
                        vp[:, 0:H].rearrange("p (n d) -> p n d", d=HD),
                        e_b, Op.mult,
                    )
                    # W[t, s] = (start_s <= t) & (t < end_s)
                    lt = asml.tile([128, SPC], f32, tag="uexp", name="lt")
                    nc.vector.tensor_scalar(
                        lt[:], endsB[:], iota_sb[:, t:t + 1], None, Op.is_gt)
                    nc.vector.scalar_tensor_tensor(
                        wt[:, t, :], startsB[:], iota_sb[:, t:t + 1], lt[:],
                        Op.is_le, Op.mult)

                # Z[n, s] then u = 1/max(Z, tiny)
                zp = psC.tile([NH, SPC], f32, tag="small")
                for t in range(TC):
                    nc.tensor.matmul(
                        zp[:], ve[:, t, H:VW], wt[:, t, :],
                        start=(t == 0), stop=(t == TC - 1))
                zc = asml.tile([NH, SPC], f32, tag="zc")
                nc.vector.tensor_scalar(zc[:], zp[:], 1e-6, None, Op.max)
                uf = asml.tile([NH, SPC], f32, tag="uexp", name="uf")
                us = asml.tile([NH, SPC], f32, tag="sq", name="us")
                nc.vector.reciprocal_approx_accurate(uf[:], zc[:], us[:])
                u_sb = asml.tile([NH, SPC], f32r, tag="u")
                with nc.allow_low_precision(reason="f32r is fp32-width"):
                    nc.vector.tensor_copy(u_sb[:], uf[:])

                # ctxN[h', s] = (sum_t ve[t, h'] * W[t, s]) * u[head(h'), s]
                for c in range(HC):
                    up = psU.tile([128, SPC], f32, tag="up")
                    nc.tensor.matmul(
                        up[:], ssel_sb[:, c * 128:(c + 1) * 128], u_sb[:],
                        start=True, stop=True)
                    uexp = asml.tile([128, SPC], f32, tag="uexp")
                    nc.scalar.activation(uexp[:], up[:], Act.Identity)
                    cp_ = psB.tile([128, SPC], f32, tag="mm512")
                    for t in range(TC):
                        nc.tensor.matmul(
                            cp_[:], ve[:, t, c * 128:(c + 1) * 128],
                            wt[:, t, :],
                            start=(t == 0), stop=(t == TC - 1))
                    nc.vector.tensor_tensor(ctxN[:, c, :], cp_[:], uexp[:], Op.mult)

                # out_proj (centered weights) + LN1, feature-major
                ycs = ap_.tile([128, HC, SPC], f32)
                varp = psC.tile([1, SPC], f32, tag="small")
                for m in range(HC):
                    aop = psB.tile([128, SPC], f32, tag="mm512")
                    for c in range(HC):
                        nc.tensor.matmul(
                            aop[:], wout_sb[:, c, m * 128:(m + 1) * 128],
                            ctxN[:, c, :],
                            start=(c == 0), stop=(c == HC - 1))
                    # yc = aop + centered bias (per-partition) on ACT
                    nc.scalar.activation(ycs[:, m, :], aop[:], Act.Identity,
                                         bias=b1ccol[:, m:m + 1])
                for m in range(HC):
                    sq = asml.tile([128, SPC], f32r, tag="sq")
                    with nc.allow_low_precision(reason="f32r is fp32-width"):
                        nc.vector.tensor_tensor(sq[:], ycs[:, m, :],
                                                ycs[:, m, :], Op.mult)
                    nc.tensor.matmul(
                        varp[:], onescol[:], sq[:],
                        start=(m == 0), stop=(m == HC - 1))
                sd = asml.tile([1, SPC], f32, tag="sd")
                nc.scalar.activation(sd[:], varp[:], Act.Sqrt,
                                     bias=eps1[:], scale=1.0 / H)
                rf = asml.tile([1, SPC], f32, tag="uexp", name="rf")
                rs_ = asml.tile([1, SPC], f32, tag="sq", name="rs_")
                nc.vector.reciprocal_approx_accurate(rf[:], sd[:], rs_[:])
                rstd = asml.tile([1, SPC], f32r, tag="rstd")
                with nc.allow_low_precision(reason="f32r is fp32-width"):
                    nc.vector.tensor_copy(rstd[:], rf[:])
                rp = psC.tile([128, SPC], f32, tag="small")
                nc.tensor.matmul(rp[:], ones1[:], rstd[:],
                                 start=True, stop=True)
                for m in range(HC):
                    if gb_identity:
                        with nc.allow_low_precision(reason="fp32-width"):
                            nc.vector.tensor_tensor(x1T[:, m, :], ycs[:, m, :],
                                                    rp[:], Op.mult)
                    else:
                        tmp = asml.tile([128, SPC], f32, tag="uexp", name="tmp")
                        nc.vector.tensor_tensor(tmp[:], ycs[:, m, :], rp[:],
                                                Op.mult)
                        # x1 = tmp * g + b (per-partition scalars) on ACT
                        nc.scalar.activation(x1T[:, m, :], tmp[:], Act.Identity,
                                             scale=gcol[:, m:m + 1],
                                             bias=bcol[:, m:m + 1])
                    nc.vector.tensor_copy(x1Tb[:, m, :], x1T[:, m, :].bitcast(f32))

            # ---------------- FFN + LN2 (h2 span-major) ----------------
            with (
                tc.tile_pool(name="ffn", bufs=1) as fp_,
                tc.tile_pool(name="outp", bufs=4) as op_,
            ):
                gB = fp_.tile([128, H], f32)    # gamma broadcast along partitions
                nc.gpsimd.dma_start(gB[:], gco.unsqueeze(0).broadcast_to([128, H]))
                bB = fp_.tile([128, H], f32)
                nc.gpsimd.dma_start(bB[:], bco.unsqueeze(0).broadcast_to([128, H]))
                b2B = fp_.tile([128, H], f32)
                nc.gpsimd.dma_start(b2B[:], b2r.unsqueeze(0).broadcast_to([128, H]))

                h1s = fp_.tile([128, IC, SPC], bf16)
                for g in range(3, NG):  # rest of W2 lands during FFN1
                    nc.scalar.dma_start(w2g[g][:], w2t[g])
                psZcm = tc.tile_pool(name="psZA", bufs=1, space="PSUM")
                psZ = psZcm.__enter__()  # wave-A banks held alongside psH
                with tc.tile_pool(name="psH", bufs=3, space="PSUM") as psH:
                    for i in range(IC):
                        g, l = divmod(i, WG)
                        if l == 0 and g >= 3:  # stream remaining weight groups
                            w1g[g % 3] = w1p.tile([128, HC, WG * 128], bf16,
                                                  tag="w1", name=f"w1g{g}")
                            nc.sync.dma_start(w1g[g % 3][:], w1t[g])
                        w1 = w1g[g % 3]
                        h1p = psH.tile([128, SPC], f32, tag="h1p")
                        for c in range(HC):
                            nc.tensor.matmul(
                                h1p[:], w1[:, c, l * 128:(l + 1) * 128],
                                x1Tb[:, c, :],
                                start=(c == 0), stop=(c == HC - 1))
                        nc.scalar.activation(h1s[:, i, :], h1p[:], Act.Relu,
                                             bias=b1col[:, i:i + 1])

                def residual_first(sc, h2p):
                    # residual: += x1 (identity matmuls transpose x1T blocks).
                    # Runs BEFORE the W2 accumulation (start=True) so no
                    # PE work remains between the last W2 matmul and LN2.
                    for hc in range(HC):
                        # start=True once per PSUM bank (clears has_written)
                        nc.tensor.matmul(
                            h2p[:, hc * 128:(hc + 1) * 128],
                            x1T[:, hc, sc * 128:(sc + 1) * 128],
                            i128_sb[:],
                            start=(hc == 0 or hc == 4), stop=False)

                def ln2_stats(sc, h2p):
                    z = op_.tile([128, H], f32, tag="z", name=f"z{sc}")
                    nc.vector.tensor_tensor(z[:], h2p[:], b2B[:], Op.add)
                    stats = op_.tile([128, 3, 6], f32, tag="stats",
                                     name=f"st{sc}")
                    zg = z[:].rearrange("p (g d) -> p g d", g=3)
                    for g in range(3):
                        nc.vector.bn_stats(stats[:, g, :], zg[:, g, :])
                    mv = op_.tile([128, 2], f32, tag="mv", name=f"mv{sc}")
                    nc.vector.bn_aggr(mv[:], stats[:])
                    return z, mv

                def ln2_sqrt(sc, mv):
                    sd2 = op_.tile([128, 1], f32, tag="sd2", name=f"sd2{sc}")
                    nc.scalar.activation(sd2[:], mv[:, 1:2], Act.Sqrt,
                                         bias=eps128[:])
                    rstd2 = op_.tile([128, 1], f32, tag="rstd2",
                                     name=f"rs2{sc}")
                    nc.vector.reciprocal(rstd2[:], sd2[:])
                    negmr = op_.tile([128, 1], f32, tag="negmr",
                                     name=f"nm{sc}")
                    nc.vector.scalar_tensor_tensor(
                        negmr[:], mv[:, 0:1], -1.0, rstd2[:], Op.mult, Op.mult)
                    return rstd2, negmr

                def ln2_norm(sc, z, rstd2, negmr):
                    if gb_identity:
                        # fold the mask into the rstd/bias scalars: one ACT op
                        rsm = op_.tile([128, 1], f32, tag="rsm", name=f"rsm{sc}")
                        nc.vector.tensor_scalar(
                            rsm[:], rstd2[:], maskcol[:, sc:sc + 1], None,
                            Op.mult)
                        nmm = op_.tile([128, 1], f32, tag="nmm", name=f"nmm{sc}")
                        nc.vector.tensor_scalar(
                            nmm[:], negmr[:], maskcol[:, sc:sc + 1], None,
                            Op.mult)
                        o = op_.tile([128, H], f32, tag="o", name=f"o{sc}")
                        nc.scalar.activation(o[:], z[:], Act.Identity,
                                             scale=rsm[:], bias=nmm[:])
                    else:
                        zn = op_.tile([128, H], f32, tag="zn", name=f"zn{sc}")
                        nc.scalar.activation(zn[:], z[:], Act.Identity,
                                             scale=rstd2[:], bias=negmr[:])
                        zn2 = op_.tile([128, H], f32, tag="zn2", name=f"zn2{sc}")
                        nc.vector.tensor_tensor(zn2[:], zn[:], gB[:], Op.mult)
                        zn3 = op_.tile([128, H], f32, tag="zn3", name=f"zn3{sc}")
                        nc.vector.tensor_tensor(zn3[:], zn2[:], bB[:], Op.add)
                        o = op_.tile([128, H], f32, tag="o", name=f"o{sc}")
                        nc.scalar.activation(o[:], zn3[:], Act.Identity,
                                             scale=maskcol[:, sc:sc + 1],
                                             bias=zero128[:])
                    nc.sync.dma_start(out[sc * 128:(sc + 1) * 128, :], o[:])

                # two waves of 2 span-chunks: wave A's matmuls interleave with
                # the FFN1 tail (4 PSUM banks fit alongside psH), and wave A's
                # LN2 overlaps wave B's matmuls
                psZBcm = tc.tile_pool(name="psZB", bufs=1, space="PSUM")
                psZB = psZBcm.__enter__()
                for wave in range(2):
                    scs = (wave * 2, wave * 2 + 1)
                    pool = psZ if wave == 0 else psZB
                    h2ps = {sc: pool.tile([128, H], f32, tag=f"h2_{sc}",
                                          name=f"h2p{sc}") for sc in scs}
                    for sc in scs:
                        residual_first(sc, h2ps[sc])
                    for i in range(IC):
                        g, l = divmod(i, WG)
                        w2 = w2g[g]
                        for sc in scs:
                            for lo, hi in ((0, 512), (512, H)):
                                nc.tensor.matmul(
                                    h2ps[sc][:, lo:hi],
                                    h1s[:, i, sc * 128:(sc + 1) * 128],
                                    w2[:, l, lo:hi],
                                    start=False, stop=(i == IC - 1))
                    zmv = {sc: ln2_stats(sc, h2ps[sc]) for sc in scs}
                    rn = {sc: ln2_sqrt(sc, zmv[sc][1]) for sc in scs}
                    for sc in scs:
                        ln2_norm(sc, zmv[sc][0], rn[sc][0], rn[sc][1])
                psZBcm.__exit__(None, None, None)
                psZcm.__exit__(None, None, None)
    nc.compile()
    return nc


def _host_prepare(inputs):
    """Host-side packing: tiny index/weight reshapes, no heavy math."""
    import ml_dtypes
    bf = ml_dtypes.bfloat16

    tr = np.asarray(inputs["token_reps"], dtype=np.float32)
    span_ids = np.asarray(inputs["span_ids"]).astype(np.int64)
    masks = np.asarray(inputs["span_masks"]).astype(np.float32)
    pe = np.asarray(inputs["pe"], dtype=np.float32)
    q0 = np.asarray(inputs["dummy_query"], dtype=np.float32)
    in_w = np.asarray(inputs["in_proj_w"], dtype=np.float32)
    in_b = np.asarray(inputs["in_proj_b"], dtype=np.float32)
    wo = np.asarray(inputs["out_proj_w"], dtype=np.float32)
    bo = np.asarray(inputs["out_proj_b"], dtype=np.float32)
    g = np.asarray(inputs["norm_g"], dtype=np.float32)
    bb = np.asarray(inputs["norm_b"], dtype=np.float32)
    w1 = np.asarray(inputs["ffn_w1"], dtype=np.float32)
    b1 = np.asarray(inputs["ffn_b1"], dtype=np.float32)
    w2 = np.asarray(inputs["ffn_w2"], dtype=np.float32)
    b2 = np.asarray(inputs["ffn_b2"], dtype=np.float32)

    Wq, Wk, Wv = in_w[0:H], in_w[H:2 * H], in_w[2 * H:3 * H]
    bq, bk, bv = in_b[0:H], in_b[H:2 * H], in_b[2 * H:3 * H]

    q = q0 @ Wq.T + bq
    qs = (q / np.sqrt(HD)).astype(np.float32)
    wq2 = np.stack([qs[n * HD:(n + 1) * HD] @ Wk[n * HD:(n + 1) * HD]
                    for n in range(NH)])                      # (12, 768)
    constv = np.array([qs[n * HD:(n + 1) * HD] @ bk[n * HD:(n + 1) * HD]
                       for n in range(NH)], dtype=np.float32)

    wvl = np.concatenate([Wv.T, wq2.T], axis=1).astype(np.float32)   # (768, 780)
    bvl = np.concatenate([bv, constv])[None, :].astype(np.float32)   # (1, 780)

    wout_c = wo - wo.mean(axis=0, keepdims=True)
    wout_ct = np.ascontiguousarray(wout_c.T).astype(np.float32)      # (768, 768)
    b1c_full = bo + q0
    b1c = (b1c_full - b1c_full.mean()).astype(np.float32)

    ssel = np.zeros((NH, H), dtype=np.float32)
    for n in range(NH):
        ssel[n, n * HD:(n + 1) * HD] = 1.0
    iota = (np.arange(128, dtype=np.float32)[:, None]
            + 128.0 * np.arange(TC, dtype=np.float32)[None, :])
    iota = np.ascontiguousarray(iota)
    i128 = np.eye(128, dtype=np.float32)

    # packed bf16 FFN weights:
    # w1t[g, p, c, l*128+n] = W1.T[c*128+p, (g*WG+l)*128+n]
    w1T = w1.T.reshape(HC, 128, NG, WG * 128)          # (c, p, g, l*128+n)
    w1tp = np.ascontiguousarray(
        w1T.transpose(2, 1, 0, 3)).astype(bf)          # (g, p, c, 512)
    # w2t[g, p, l, h] = W2.T[(g*WG+l)*128+p, h]
    w2T = w2.T.reshape(NG, WG, 128, H)                 # (g, l, p, h)
    w2tp = np.ascontiguousarray(
        w2T.transpose(0, 2, 1, 3)).astype(bf)          # (g, p, l, h)

    x = tr + pe[None, :T]                              # (B, T, H)
    xTs = [np.ascontiguousarray(x[b].T).astype(np.float32) for b in range(B)]

    starts_all = span_ids[..., 0].astype(np.float32)                      # (B, S)
    lens_all = (span_ids[..., 1] - span_ids[..., 0]).astype(np.float32) * masks
    ends_all = starts_all + lens_all

    shared = dict(wvl=wvl.astype(bf), bvl=bvl.astype(bf), iota=iota,
                  ssel=ssel, wout=wout_ct.astype(bf),
                  b1c=b1c, gco=g, bco=bb, w1t=w1tp, b1r=b1, w2t=w2tp, b2r=b2,
                  i128=i128, onesv=np.ones(128, dtype=np.float32),
                  onesb=np.ones(128, dtype=bf))
    in_maps = []
    for core in range(NCORES):
        b = core // (NCORES // B)
        s0 = (core % (NCORES // B)) * SPC
        m = dict(shared)
        m["xT"] = xTs[b].astype(bf)
        m["starts"] = np.ascontiguousarray(starts_all[b, s0:s0 + SPC][None, :])
        m["ends"] = np.ascontiguousarray(ends_all[b, s0:s0 + SPC][None, :])
        m["maskc"] = np.ascontiguousarray(masks[b, s0:s0 + SPC])
        in_maps.append(m)
    return in_maps


def kernel(**inputs) -> np.ndarray:
    global _COMPILED
    from concourse.bass_utils import run_bass_kernel_spmd

    if _COMPILED is None:
        gbi = (np.allclose(np.asarray(inputs["norm_g"], dtype=np.float32), 1.0)
               and np.allclose(np.asarray(inputs["norm_b"], dtype=np.float32),
                               0.0))
        _COMPILED = _build(gb_identity=gbi)
    nc = _COMPILED
    in_maps = _host_prepare(inputs)
    res = run_bass_kernel_spmd(nc, in_maps, core_ids=list(range(NCORES)))
    outs = [res.results[i]["out"] for i in range(NCORES)]
    full = np.concatenate(outs, axis=0).reshape(B, S, H)
    return full.astype(np.float32)
